# revision 13
# baseline (speedup 1.0000x reference)
"""Denoising bilateral-grid kernel on 8 Trainium2 NeuronCores (Bass/Tile).

Sharding: batch(2) x H-bands(4) = 8 cores. Each core computes a 128-row
output band from a 200-row zero-padded window (halo 36 = 30 chrom-blur
+ 6 lum-blur). Out-of-image rows are poisoned (z = -1e4) so they splat
nothing, reproducing the reference's zero-padded grid.

Math (validated vs reference):
  splat profile  S~_d(z) = C * exp(-alpha (z-d)^2)    [= Fr applied to the
                 2-sparse splat hat, approximated analytically; alpha, C
                 derived from filter_r at runtime]
  grid planes    g_dc = val_c * S~_d  (+ weight plane S~_d)
  spatial blur   TH/TW Toeplitz matmuls on TensorE in bf16
  slice          out_c = sum_d hat_d(z') * blur2d(g_dc)  (exact hats)
  result         num / max(den, 1e-8)

Perf structure: x-part tensors consolidated into single wide tiles so
elementwise work runs as few wide instructions; slice multiplies read
conv2 output directly from PSUM (no eviction copies); conv1 PSUM packs
4 chunks per 2-bank tile so eviction is one strided cast.
"""
import sys
sys.path.insert(0, '/opt/trn_rl_repo')
import math
import numpy as np

import concourse.bass as bass
import concourse.tile as tile
from concourse import mybir
from concourse.bass_utils import run_bass_kernel_spmd
from concourse.masks import make_identity
from contextlib import ExitStack

F32 = mybir.dt.float32
F32R = mybir.dt.float32r
BF16 = mybir.dt.bfloat16

D = 32
H = W = 512
BATCH = 2
N_BANDS = 4
BAND = H // N_BANDS          # 128
HALO_C = 30                  # chrom spatial radius (61 taps)
HALO_L = HALO_C + 6          # + lum spatial radius (13 taps) = 36
RL = BAND + 2 * HALO_L       # 200 lum window rows
RC = BAND + 2 * HALO_C       # 188 chrom window rows
NPAD = 256                   # moving-dim pad

_RGB2YUV = np.array([[0.299, 0.587, 0.114],
                     [-0.14713, -0.28886, 0.436],
                     [0.615, -0.51499, -0.10001]], dtype=np.float32)
_YUV2RGB = np.linalg.inv(_RGB2YUV).astype(np.float32)

POISON = -1.0e4


def _toeplitz(k, n_in, n_out, off):
    """T[i, o] = k[i - o - off + r]; out col o corresponds to input row o+off."""
    k = np.asarray(k, np.float64)
    r = (len(k) - 1) // 2
    ii = np.arange(n_in)[:, None]
    oo = np.arange(n_out)[None, :]
    d = ii - (oo + off) + r
    T = np.zeros((n_in, n_out), np.float32)
    m = (d >= 0) & (d < len(k))
    T[m] = k[d[m]].astype(np.float32)
    return T


def _ap(t, offset, dims):
    """Manual AP over tile t: partition dim from t, free dims = dims."""
    a = t[:]
    return bass.AP(tensor=a.tensor, offset=a.offset + offset,
                   ap=[a.ap[0]] + [list(d) for d in dims])


# ---------------------------------------------------------------------------
# device program
# ---------------------------------------------------------------------------

def build_nc(waitsplit=True):
    import time as _time
    _t0 = _time.time()
    print("build_nc: start", flush=True)
    nc = bass.Bass()

    img = nc.declare_dram_parameter("img", [3, RL, W], F32, isOutput=False)
    thl = nc.declare_dram_parameter("thl", [RL, NPAD], F32, isOutput=False)
    twl = nc.declare_dram_parameter("twl", [W, W], F32, isOutput=False)
    thc = nc.declare_dram_parameter("thc", [RC, BAND], F32, isOutput=False)
    twc = nc.declare_dram_parameter("twc", [W, W], F32, isOutput=False)
    consts = nc.declare_dram_parameter("consts", [128, 104], F32, isOutput=False)
    poil = nc.declare_dram_parameter("poil", [RL, 1], F32, isOutput=False)
    poic = nc.declare_dram_parameter("poic", [128, RC], F32, isOutput=False)
    out = nc.declare_dram_parameter("out", [3, BAND, W], F32, isOutput=True)

    C_EXPB_L, C_EXPB_C, C_HATB, C_AL, C_AC, C_ONE, C_EPS = 0, 32, 64, 96, 97, 98, 99

    R0 = 128          # rows in lum tile0
    R1 = RL - 128     # 72 rows in lum tile1
    CA0 = 6           # chrom rows start within lum window

    AL = mybir.AluOpType
    AF = mybir.ActivationFunctionType

    with tile.TileContext(nc) as tc, ExitStack() as top:
        P = top.enter_context(tc.tile_pool(name="persist", bufs=1))

        # ---- constants / inputs ----
        cst = P.tile([128, 104], F32, tag="cst", name="cst")
        nc.sync.dma_start(out=cst[:], in_=consts[:])
        ident = P.tile([128, 128], F32, tag="ident", name="ident")
        make_identity(nc, ident[:])

        rgb = []
        for ch in range(3):
            t0 = P.tile([R0, W], F32, tag=f"rgb{ch}0", name=f"rgb{ch}0")
            t1 = P.tile([R1, W], F32, tag=f"rgb{ch}1", name=f"rgb{ch}1")
            nc.sync.dma_start(out=t0[:], in_=img[ch, 0:R0, :])
            nc.sync.dma_start(out=t1[:], in_=img[ch, R0:RL, :])
            rgb.append((t0, t1))
        poil_t0 = P.tile([R0, 1], F32, tag="poil0", name="poil0")
        poil_t1 = P.tile([R1, 1], F32, tag="poil1", name="poil1")
        nc.sync.dma_start(out=poil_t0[:], in_=poil[0:R0, :])
        nc.sync.dma_start(out=poil_t1[:], in_=poil[R0:RL, :])
        poic_t = P.tile([128, RC], F32, tag="poic", name="poic")
        nc.sync.dma_start(out=poic_t[:], in_=poic[:])

        # Toeplitz constants -> bf16 (DVE cast; DMA-produced f32 first)
        thl_f = [P.tile([R0, NPAD], F32, tag="thlf0", name="thlf0"),
                 P.tile([R1, NPAD], F32, tag="thlf1", name="thlf1")]
        nc.sync.dma_start(out=thl_f[0][:], in_=thl[0:R0, :])
        nc.sync.dma_start(out=thl_f[1][:], in_=thl[R0:RL, :])
        thl_r = [P.tile([R0, NPAD], BF16, tag="thlr0", name="thlr0"),
                 P.tile([R1, NPAD], BF16, tag="thlr1", name="thlr1")]
        nc.vector.tensor_copy(thl_r[0][:], thl_f[0][:])
        nc.vector.tensor_copy(thl_r[1][:], thl_f[1][:])

        twl_f = [P.tile([128, W], F32, tag=f"twlf{i}", name=f"twlf{i}") for i in range(4)]
        twl_r = [P.tile([128, W], BF16, tag=f"twlr{i}", name=f"twlr{i}") for i in range(4)]
        for i in range(4):
            nc.sync.dma_start(out=twl_f[i][:], in_=twl[128 * i:128 * (i + 1), :])
            nc.vector.tensor_copy(twl_r[i][:], twl_f[i][:])
        twc_f = [P.tile([128, W], F32, tag=f"twcf{i}", name=f"twcf{i}") for i in range(4)]
        twc_r = [P.tile([128, W], BF16, tag=f"twcr{i}", name=f"twcr{i}") for i in range(4)]
        for i in range(4):
            nc.sync.dma_start(out=twc_f[i][:], in_=twc[128 * i:128 * (i + 1), :])
            nc.vector.tensor_copy(twc_r[i][:], twc_f[i][:])
        thc_f = [P.tile([128, BAND], F32, tag="thcf0", name="thcf0"),
                 P.tile([RC - 128, BAND], F32, tag="thcf1", name="thcf1")]
        nc.sync.dma_start(out=thc_f[0][:], in_=thc[0:128, :])
        nc.sync.dma_start(out=thc_f[1][:], in_=thc[128:RC, :])
        thc_r = [P.tile([128, BAND], BF16, tag="thcr0", name="thcr0"),
                 P.tile([RC - 128, BAND], BF16, tag="thcr1", name="thcr1")]
        nc.vector.tensor_copy(thc_r[0][:], thc_f[0][:])
        nc.vector.tensor_copy(thc_r[1][:], thc_f[1][:])

        # ---- P1: YUV + z (y-part) ----
        lum = [P.tile([R0, W], F32, tag="lum0", name="lum0"), P.tile([R1, W], F32, tag="lum1", name="lum1")]
        uch = [P.tile([R0, W], F32, tag="u0", name="u0"), P.tile([R1, W], F32, tag="u1", name="u1")]
        vch = [P.tile([R0, W], F32, tag="v0", name="v0"), P.tile([R1, W], F32, tag="v1", name="v1")]
        for i in range(2):
            r_t, g_t, b_t = rgb[0][i], rgb[1][i], rgb[2][i]
            for dst, row in ((lum, 0), (uch, 1), (vch, 2)):
                c0, c1, c2 = (float(_RGB2YUV[row, 0]), float(_RGB2YUV[row, 1]),
                              float(_RGB2YUV[row, 2]))
                nc.vector.tensor_scalar_mul(dst[i][:], r_t[:], c0)
                nc.vector.scalar_tensor_tensor(dst[i][:], g_t[:], c1, dst[i][:],
                                               AL.mult, AL.add)
                nc.vector.scalar_tensor_tensor(dst[i][:], b_t[:], c2, dst[i][:],
                                               AL.mult, AL.add)

        z = [P.tile([R0, W], F32, tag="z0", name="z0"), P.tile([R1, W], F32, tag="z1", name="z1")]
        za = [P.tile([R0, W], F32, tag="za0", name="za0"), P.tile([R1, W], F32, tag="za1", name="za1")]
        za2 = [P.tile([R0, W], F32, tag="za20", name="za20"), P.tile([R1, W], F32, tag="za21", name="za21")]
        for i, poi_t in ((0, poil_t0), (1, poil_t1)):
            n = R0 if i == 0 else R1
            nc.vector.tensor_scalar(z[i][:], lum[i][:], 0.0, 1.0, AL.max, AL.min)
            nc.vector.tensor_scalar(z[i][:], z[i][:], float(D - 1), poi_t[:, 0:1],
                                    AL.mult, AL.add)
            # za = alpha_l * z ; za2 = alpha_l * z^2
            nc.vector.tensor_single_scalar(za[i][:], z[i][:], cst[:n, C_AL:C_AL + 1], AL.mult)
            nc.vector.tensor_mul(za2[i][:], za[i][:], z[i][:])

        lum_bf = [P.tile([R0, W], BF16, tag="lumbf0", name="lumbf0"),
                  P.tile([R1, W], BF16, tag="lumbf1", name="lumbf1")]
        nc.vector.tensor_copy(lum_bf[0][:], lum[0][:])
        nc.vector.tensor_copy(lum_bf[1][:], lum[1][:])

        # ---- transposes: zT, U_x, V_x (x-part, chrom rows) -> WIDE tiles ----
        zT = P.tile([128, 4 * RC], F32, tag="zT", name="zT")
        u_x = P.tile([128, 4 * RC], BF16, tag="ux", name="ux")
        v_x = P.tile([128, 4 * RC], BF16, tag="vx", name="vx")

        with ExitStack() as tctx:
            tp_ps = tctx.enter_context(tc.tile_pool(name="tp_ps", bufs=2, space="PSUM"))

            def transpose_chrom_rows(src_pair, dst_wide):
                for xc in range(4):
                    ps = tp_ps.tile([128, RL], F32, tag="tpps", name="tpps")
                    nc.tensor.transpose(ps[:, 0:R0],
                                        src_pair[0][:, 128 * xc:128 * (xc + 1)],
                                        ident[:])
                    nc.tensor.transpose(ps[:, R0:RL],
                                        src_pair[1][:, 128 * xc:128 * (xc + 1)],
                                        ident[0:R1, 0:R1])
                    dst = dst_wide[:, xc * RC:(xc + 1) * RC]
                    if xc % 2 == 0:
                        nc.vector.tensor_copy(dst, ps[:, CA0:CA0 + RC])
                    else:
                        nc.scalar.copy(dst, ps[:, CA0:CA0 + RC])

            transpose_chrom_rows(z, zT)
            transpose_chrom_rows(uch, u_x)
            transpose_chrom_rows(vch, v_x)

        # ---- P2: LUM bins ----
        # numden layout: [xc][ci=2][RC]  (ci=0 num, ci=1 den)
        numden_l = P.tile([128, 4 * 2 * RC], F32, tag="numden", name="numden")
        nc.vector.memset(numden_l[:], 0.0)

        with ExitStack() as lctx:
            LP = lctx.enter_context(tc.tile_pool(name="lum_loop", bufs=2))
            # conv1 psum: one [128, 1024] f32 (2 banks) per ci; 4 chunks at 256-slots
            ps1 = lctx.enter_context(tc.tile_pool(name="l_ps1", bufs=2, space="PSUM"))
            # conv2 psum: one [128, 1024] f32 (2 banks) per xo-pair; 2 per d
            ps2 = lctx.enter_context(tc.tile_pool(name="l_ps2", bufs=2, space="PSUM"))

            for d in range(D):
                # splat profile: exp(2d*za - za2 + expb_l[d])
                u2 = [LP.tile([R0, W], F32, tag="u2_0", name="u2_0"),
                      LP.tile([R1, W], F32, tag="u2_1", name="u2_1")]
                prof = [LP.tile([R0, W], BF16, tag="prof0", name="prof0"),
                        LP.tile([R1, W], BF16, tag="prof1", name="prof1")]
                gy = [LP.tile([R0, W], BF16, tag="gy0", name="gy0"),
                      LP.tile([R1, W], BF16, tag="gy1", name="gy1")]
                nc.vector.scalar_tensor_tensor(u2[0][:], za[0][:], float(2.0 * d),
                                               za2[0][:], AL.mult, AL.subtract)
                nc.vector.scalar_tensor_tensor(u2[1][:], za[1][:], float(2.0 * d),
                                               za2[1][:], AL.mult, AL.subtract)
                for i in range(2):
                    n = R0 if i == 0 else R1
                    nc.scalar.activation(prof[i][:], u2[i][:], AF.Exp,
                                         bias=cst[:n, C_EXPB_L + d:C_EXPB_L + d + 1],
                                         scale=1.0)
                    if i == 0:
                        nc.gpsimd.tensor_mul(gy[i][:], lum_bf[i][:], prof[i][:])
                    else:
                        nc.vector.tensor_mul(gy[i][:], lum_bf[i][:], prof[i][:])

                # conv1: H-conv fused transpose -> packed psum, one strided cast per ci
                # c1p layout: [xi][ci=2][RC] bf16
                c1p = LP.tile([128, 4 * 2 * RC], BF16, tag="l_c1p", name="l_c1p")
                for ci, ch in enumerate((gy, prof)):
                    pst = ps1.tile([128, 1024], F32, tag="l_c1ps", name="l_c1ps")
                    for xc in range(4):
                        dst = pst[:, xc * 256:xc * 256 + RC]
                        nc.tensor.matmul(dst, ch[0][:, 128 * xc:128 * (xc + 1)],
                                         thl_r[0][:, 0:RC], start=True, stop=False)
                        nc.tensor.matmul(dst, ch[1][:, 128 * xc:128 * (xc + 1)],
                                         thl_r[1][:, 0:RC], start=False, stop=True)
                    src = _ap(pst, 0, [[256, 4], [1, RC]])
                    dst = _ap(c1p, ci * RC, [[2 * RC, 4], [1, RC]])
                    if ci == 0:
                        nc.vector.tensor_copy(dst, src)
                    else:
                        nc.scalar.copy(dst, src)

                # conv2: W-conv (x-part); xo pairs into [128,1024] psum (2 banks)
                pst2 = []
                for pair in range(2):
                    pt = ps2.tile([128, 1024], F32, tag="l_c2ps", name="l_c2ps")
                    pst2.append(pt)
                    for sub in range(2):
                        xo = 2 * pair + sub
                        dst = pt[:, sub * 512:sub * 512 + 2 * RC]
                        first = True
                        xis = [x for x in (xo - 1, xo, xo + 1) if 0 <= x <= 3]
                        for xi in xis:
                            nc.tensor.matmul(
                                dst, twl_r[xi][:, 128 * xo:128 * (xo + 1)],
                                c1p[:, xi * 2 * RC:(xi + 1) * 2 * RC],
                                start=first, stop=(xi == xis[-1]))
                            first = False

                # slice: hat_d(zT) wide; mul reads PSUM directly; add on gpsimd
                hat = LP.tile([128, 4 * RC], F32, tag="l_hat", name="l_hat")
                hu = LP.tile([128, 4 * RC], F32, tag="l_hu", name="l_hu")
                nc.scalar.activation(hu[:], zT[:], AF.Abs,
                                     bias=cst[:, C_HATB + d:C_HATB + d + 1], scale=1.0)
                nc.scalar.activation(hat[:], hu[:], AF.Relu,
                                     bias=cst[:, C_ONE:C_ONE + 1], scale=-1.0)
                for pair in range(2):
                    pr = LP.tile([128, 2 * 2 * RC], F32, tag=f"l_pr{pair}", name=f"l_pr{pair}")
                    hat_b = _ap(hat, pair * 2 * RC, [[RC, 2], [0, 2], [1, RC]])
                    ps_r = _ap(pst2[pair], 0, [[512, 2], [RC, 2], [1, RC]])
                    nc.vector.tensor_mul(pr[:], hat_b, ps_r)
                    dst = numden_l[:, pair * 2 * 2 * RC:(pair + 1) * 2 * 2 * RC]
                    nc.gpsimd.tensor_add(dst, dst, pr[:])

        # ---- out_lum = num/max(den,eps); zc = clip(OL)*31 + poison ----
        # numden layout [xc][num RC | den RC]; build wide ol/zc [xc][RC]
        den_ap = _ap(numden_l, RC, [[2 * RC, 4], [1, RC]])
        nc.vector.tensor_single_scalar(den_ap, den_ap, cst[:, C_EPS:C_EPS + 1], AL.max)
        denr = P.tile([128, 4 * RC], F32, tag="denr", name="denr")
        nc.vector.reciprocal(denr[:], den_ap)
        ol = P.tile([128, 4 * RC], F32, tag="ol", name="ol")
        num_ap = _ap(numden_l, 0, [[2 * RC, 4], [1, RC]])
        nc.vector.tensor_mul(ol[:], num_ap, denr[:])
        zc = P.tile([128, 4 * RC], F32, tag="zc", name="zc")
        zca = P.tile([128, 4 * RC], F32, tag="zca", name="zca")
        zca2 = P.tile([128, 4 * RC], F32, tag="zca2", name="zca2")
        nc.vector.tensor_scalar(zc[:], ol[:], 0.0, 1.0, AL.max, AL.min)
        nc.vector.tensor_scalar_mul(zc[:], zc[:], float(D - 1))
        poic_b = _ap(poic_t, 0, [[0, 4], [1, RC]])
        nc.vector.tensor_add(zc[:], zc[:], poic_b)
        nc.vector.tensor_single_scalar(zca[:], zc[:], cst[:, C_AC:C_AC + 1], AL.mult)
        nc.vector.tensor_mul(zca2[:], zca[:], zc[:])

        # zcT: y-part zc at band rows (chrom window rows 30..158)
        zcT = P.tile([128, W], F32, tag="zcT", name="zcT")
        olb = P.tile([128, W], F32, tag="olb", name="olb")
        with ExitStack() as tctx2:
            tp2 = tctx2.enter_context(tc.tile_pool(name="tp_ps2", bufs=2, space="PSUM"))
            for xc in range(4):
                ps = tp2.tile([128, 128], F32, tag="tpps2", name="tpps2")
                nc.tensor.transpose(ps[:], zc[:, xc * RC + HALO_C:xc * RC + HALO_C + BAND], ident[:])
                nc.scalar.copy(zcT[:, 128 * xc:128 * (xc + 1)], ps[:])
                ps2b = tp2.tile([128, 128], F32, tag="tpps2", name="tpps2")
                nc.tensor.transpose(ps2b[:], ol[:, xc * RC + HALO_C:xc * RC + HALO_C + BAND], ident[:])
                nc.vector.tensor_copy(olb[:, 128 * xc:128 * (xc + 1)], ps2b[:])

        # ---- P4: CHROM bins ----
        acc_c = P.tile([128, 3 * W], F32, tag="accc", name="accc")
        nc.vector.memset(acc_c[:], 0.0)

        with ExitStack() as cctx:
            CP = cctx.enter_context(tc.tile_pool(name="ch_loop", bufs=2))
            cps1 = cctx.enter_context(tc.tile_pool(name="c_ps1", bufs=4, space="PSUM"))
            cps2 = cctx.enter_context(tc.tile_pool(name="c_ps2", bufs=1, space="PSUM"))

            YC0, YC1 = 128, RC - 128  # y chunks of the chrom window (128 + 60)
            for d in range(D):
                # wide splat: u2c/profc/gu/gv [128, 4*RC]
                u2c = CP.tile([128, 4 * RC], F32, tag="cu2", name="cu2")
                profc = CP.tile([128, 4 * RC], BF16, tag="cprof", name="cprof")
                gu = CP.tile([128, 4 * RC], BF16, tag="cgu", name="cgu")
                gv = CP.tile([128, 4 * RC], BF16, tag="cgv", name="cgv")
                nc.vector.scalar_tensor_tensor(u2c[:], zca[:], float(2.0 * d),
                                               zca2[:], AL.mult, AL.subtract)
                nc.scalar.activation(profc[:], u2c[:], AF.Exp,
                                     bias=cst[:, C_EXPB_C + d:C_EXPB_C + d + 1],
                                     scale=1.0)
                nc.vector.tensor_mul(gu[:], u_x[:], profc[:])
                nc.gpsimd.tensor_mul(gv[:], v_x[:], profc[:])

                # conv1: W-conv fused transpose -> [y, x_out 512]
                c1c = {}
                for ci, ch in enumerate((gu, gv, profc)):
                    for yc, (ys, nrows) in enumerate(((0, YC0), (YC0, YC1))):
                        ps = cps1.tile([128, W], F32, tag="c_c1ps", name="c_c1ps")
                        nc.tensor.matmul(ps[0:nrows, :], ch[:, ys:ys + nrows],
                                         twc_r[0][:], start=True, stop=False)
                        for xi in (1, 2, 3):
                            s = min(128 * xi - HALO_C, W - NPAD)
                            nc.tensor.matmul(
                                ps[0:nrows, s:s + NPAD],
                                ch[:, xi * RC + ys:xi * RC + ys + nrows],
                                twc_r[xi][:, s:s + NPAD],
                                start=False, stop=(xi == 3))
                        t = CP.tile([128, W], BF16, tag=f"c_c1_{ci}_{yc}", name=f"c_c1_{ci}_{yc}")
                        # large (yc=0, 128-row) casts mostly on scalar; small
                        # (yc=1, 60-row) mostly on vector — vector is the wall
                        if (yc == 0) == (ci < 2):
                            nc.scalar.copy(t[0:nrows, :], ps[0:nrows, :])
                        else:
                            nc.vector.tensor_copy(t[0:nrows, :], ps[0:nrows, :])
                        c1c[(ci, yc)] = t

                # conv2: H-conv y-part -> [y_out 128, x 512] x3 into one psum tile
                pt = cps2.tile([128, 3 * W], F32, tag="c_c2ps", name="c_c2ps")
                for ci in range(3):
                    dst = pt[:, ci * W:(ci + 1) * W]
                    nc.tensor.matmul(dst, thc_r[0][:], c1c[(ci, 0)][0:YC0, :],
                                     start=True, stop=False)
                    nc.tensor.matmul(dst, thc_r[1][:], c1c[(ci, 1)][0:YC1, :],
                                     start=False, stop=True)

                # slice (y-part): hat wide; mul reads PSUM; add on gpsimd
                hat = CP.tile([128, W], F32, tag="c_hat", name="c_hat")
                hu = CP.tile([128, W], F32, tag="c_hu", name="c_hu")
                nc.scalar.activation(hu[:], zcT[:], AF.Abs,
                                     bias=cst[:, C_HATB + d:C_HATB + d + 1], scale=1.0)
                nc.scalar.activation(hat[:], hu[:], AF.Relu,
                                     bias=cst[:, C_ONE:C_ONE + 1], scale=-1.0)
                hat_b = _ap(hat, 0, [[0, 3], [1, W]])
                prc = CP.tile([128, 3 * W], F32, tag="c_prc", name="c_prc")
                nc.vector.tensor_mul(prc[:], hat_b, pt[:])
                nc.gpsimd.tensor_add(acc_c[:], acc_c[:], prc[:])

        # ---- P5: output ----
        den_v = acc_c[:, 2 * W:3 * W]
        nc.vector.tensor_single_scalar(den_v, den_v, cst[:, C_EPS:C_EPS + 1],
                                       AL.max)
        nc.vector.reciprocal(den_v, den_v)
        ocu = P.tile([128, W], F32, tag="ocu", name="ocu")
        ocv = P.tile([128, W], F32, tag="ocv", name="ocv")
        nc.vector.tensor_mul(ocu[:], acc_c[:, 0:W], den_v)
        nc.vector.tensor_mul(ocv[:], acc_c[:, W:2 * W], den_v)

        for ch in range(3):
            c0, c1_, c2 = (float(_YUV2RGB[ch, 0]), float(_YUV2RGB[ch, 1]),
                           float(_YUV2RGB[ch, 2]))
            o = P.tile([128, W], F32, tag=f"outc{ch}", name=f"outc{ch}")
            nc.vector.tensor_scalar_mul(o[:], olb[:], c0)
            nc.vector.scalar_tensor_tensor(o[:], ocu[:], c1_, o[:], AL.mult, AL.add)
            nc.vector.scalar_tensor_tensor(o[:], ocv[:], c2, o[:], AL.mult, AL.add)
            nc.sync.dma_start(out=out[ch, :, :], in_=o[:])

    print(f"build_nc: traced+scheduled in {_time.time()-_t0:.1f}s", flush=True)
    if waitsplit:
        _split_multi_waits(nc)
    return nc


def _split_multi_waits(nc, max_waits=1):
    counter = [0]
    for f in nc.m.functions:
        for blk in f.blocks:
            insts = blk.instructions
            snapshot = list(insts)
            offset = 0
            for idx, inst in enumerate(snapshot):
                si = inst.sync_info
                if si is None or not si.on_wait or len(si.on_wait) <= max_waits:
                    continue
                waits = list(si.on_wait)
                keep = waits[-max_waits:]
                extra = waits[:-max_waits]
                new_insts = []
                for wcond in extra:
                    ev = mybir.InstEventSemaphore(
                        name=f"I-wsplit-{counter[0]}", ins=[], outs=[])
                    counter[0] += 1
                    ev.engine = inst.engine
                    ev.sync_info = mybir.SyncInfo(on_wait=[wcond], on_update=[])
                    new_insts.append(ev)
                inst.sync_info = mybir.SyncInfo(on_wait=keep,
                                                on_update=list(si.on_update))
                for j, ev in enumerate(new_insts):
                    insts.insert(idx + offset + j, ev)
                offset += len(new_insts)


# ---------------------------------------------------------------------------
# host side
# ---------------------------------------------------------------------------

_NC_CACHE = None


def _get_nc():
    global _NC_CACHE
    if _NC_CACHE is None:
        _NC_CACHE = build_nc()
    return _NC_CACHE


def _make_in_maps(image, f_s, f_r, f_sc, f_rc):
    image = np.ascontiguousarray(np.asarray(image, np.float32))
    f_s = np.asarray(f_s, np.float32)
    f_sc = np.asarray(f_sc, np.float32)
    f_r = np.asarray(f_r, np.float64)
    f_rc = np.asarray(f_rc, np.float64)

    thl_m = _toeplitz(f_s, RL, RC, (len(f_s) - 1) // 2)
    thl_pad = np.zeros((RL, NPAD), np.float32)
    thl_pad[:, :RC] = thl_m
    twl_m = _toeplitz(f_s, W, W, 0)
    thc_m = _toeplitz(f_sc, RC, BAND, (len(f_sc) - 1) // 2)
    twc_m = _toeplitz(f_sc, W, W, 0)

    def alpha_lnc(k):
        c = (len(k) - 1) // 2
        alpha = float(np.log(k[c] / k[c + 1]))
        lnc = float(np.log(k[c]))
        return alpha, lnc

    al, lncl = alpha_lnc(f_r)
    ac, lncc = alpha_lnc(f_rc)

    consts = np.zeros((128, 104), np.float32)
    dd = np.arange(D, dtype=np.float64)
    consts[:, 0:32] = (lncl - al * dd * dd)[None, :]
    consts[:, 32:64] = (lncc - ac * dd * dd)[None, :]
    consts[:, 64:96] = (-dd)[None, :]
    consts[:, 96] = al
    consts[:, 97] = ac
    consts[:, 98] = 1.0
    consts[:, 99] = 1e-8

    in_maps = []
    for c in range(8):
        b, t = divmod(c, N_BANDS)
        s = t * BAND
        rows = np.arange(s - HALO_L, s + BAND + HALO_L)
        valid = (rows >= 0) & (rows < H)
        win = np.zeros((3, RL, W), np.float32)
        win[:, valid] = image[b][:, rows[valid]]
        poil_v = np.where(valid, 0.0, POISON).astype(np.float32)[:, None]
        validc = valid[HALO_L - HALO_C:HALO_L - HALO_C + RC]
        poic_v = np.broadcast_to(
            np.where(validc, 0.0, POISON).astype(np.float32)[None, :],
            (128, RC)).copy()
        in_maps.append({
            "img": win, "thl": thl_pad, "twl": twl_m, "thc": thc_m,
            "twc": twc_m, "consts": consts, "poil": poil_v, "poic": poic_v,
        })
    return in_maps


def kernel(image, filter_s, filter_r, filter_s_color, filter_r_color,
           _trace=False):
    nc = _get_nc()
    in_maps = _make_in_maps(image, filter_s, filter_r, filter_s_color,
                            filter_r_color)
    res = run_bass_kernel_spmd(nc, in_maps, list(range(8)), trace=_trace)
    out = np.zeros((BATCH, 3, H, W), np.float32)
    for c in range(8):
        b, t = divmod(c, N_BANDS)
        out[b, :, t * BAND:(t + 1) * BAND, :] = res.results[c]["out"]
    if _trace:
        return out, res
    return out


# revision 14
# speedup vs baseline: 1.0691x; 1.0691x over previous
"""Denoising bilateral-grid kernel on 8 Trainium2 NeuronCores (Bass/Tile).

Sharding: batch(2) x H-bands(4) = 8 cores. Each core computes a 128-row
output band from a 200-row zero-padded window (halo 36 = 30 chrom-blur
+ 6 lum-blur). Out-of-image rows are poisoned (z = -1e4) so they splat
nothing, reproducing the reference's zero-padded grid.

Math (validated vs reference):
  splat profile  S~_d(z) = C * exp(-alpha (z-d)^2)    [= Fr applied to the
                 2-sparse splat hat, approximated analytically; alpha, C
                 derived from filter_r at runtime]
  grid planes    g_dc = val_c * S~_d  (+ weight plane S~_d)
  spatial blur   TH/TW Toeplitz matmuls on TensorE in bf16
  slice          out_c = sum_d hat_d(z') * blur2d(g_dc)  (exact hats)
  result         num / max(den, 1e-8)

Perf structure: x-part tensors consolidated into single wide tiles so
elementwise work runs as few wide instructions; slice multiplies read
conv2 output directly from PSUM (no eviction copies); conv1 PSUM packs
4 chunks per 2-bank tile so eviction is one strided cast.
"""
import sys
sys.path.insert(0, '/opt/trn_rl_repo')
import math
import numpy as np

import concourse.bass as bass
import concourse.tile as tile
from concourse import mybir
from concourse.bass_utils import run_bass_kernel_spmd
from concourse.masks import make_identity
from contextlib import ExitStack

F32 = mybir.dt.float32
F32R = mybir.dt.float32r
BF16 = mybir.dt.bfloat16

D = 32
H = W = 512
BATCH = 2
N_BANDS = 4
BAND = H // N_BANDS          # 128
HALO_C = 30                  # chrom spatial radius (61 taps)
HALO_L = HALO_C + 6          # + lum spatial radius (13 taps) = 36
RL = BAND + 2 * HALO_L       # 200 lum window rows
RC = BAND + 2 * HALO_C       # 188 chrom window rows
NPAD = 256                   # moving-dim pad

_RGB2YUV = np.array([[0.299, 0.587, 0.114],
                     [-0.14713, -0.28886, 0.436],
                     [0.615, -0.51499, -0.10001]], dtype=np.float32)
_YUV2RGB = np.linalg.inv(_RGB2YUV).astype(np.float32)

POISON = -1.0e4


def _toeplitz(k, n_in, n_out, off):
    """T[i, o] = k[i - o - off + r]; out col o corresponds to input row o+off."""
    k = np.asarray(k, np.float64)
    r = (len(k) - 1) // 2
    ii = np.arange(n_in)[:, None]
    oo = np.arange(n_out)[None, :]
    d = ii - (oo + off) + r
    T = np.zeros((n_in, n_out), np.float32)
    m = (d >= 0) & (d < len(k))
    T[m] = k[d[m]].astype(np.float32)
    return T


def _ap(t, offset, dims):
    """Manual AP over tile t: partition dim from t, free dims = dims."""
    a = t[:]
    return bass.AP(tensor=a.tensor, offset=a.offset + offset,
                   ap=[a.ap[0]] + [list(d) for d in dims])


# ---------------------------------------------------------------------------
# device program
# ---------------------------------------------------------------------------

def build_nc(waitsplit=True):
    import time as _time
    _t0 = _time.time()
    print("build_nc: start", flush=True)
    nc = bass.Bass()

    img = nc.declare_dram_parameter("img", [3, RL, W], F32, isOutput=False)
    thl = nc.declare_dram_parameter("thl", [RL, NPAD], F32, isOutput=False)
    twl = nc.declare_dram_parameter("twl", [W, W], F32, isOutput=False)
    thc = nc.declare_dram_parameter("thc", [RC, BAND], F32, isOutput=False)
    twc = nc.declare_dram_parameter("twc", [W, W], F32, isOutput=False)
    consts = nc.declare_dram_parameter("consts", [128, 104], F32, isOutput=False)
    poil = nc.declare_dram_parameter("poil", [RL, 1], F32, isOutput=False)
    poic = nc.declare_dram_parameter("poic", [128, RC], F32, isOutput=False)
    out = nc.declare_dram_parameter("out", [3, BAND, W], F32, isOutput=True)

    C_EXPB_L, C_EXPB_C, C_HATB, C_AL, C_AC, C_ONE, C_EPS = 0, 32, 64, 96, 97, 98, 99

    R0 = 128          # rows in lum tile0
    R1 = RL - 128     # 72 rows in lum tile1
    CA0 = 6           # chrom rows start within lum window

    AL = mybir.AluOpType
    AF = mybir.ActivationFunctionType

    with tile.TileContext(nc) as tc, ExitStack() as top:
        P = top.enter_context(tc.tile_pool(name="persist", bufs=1))

        # ---- constants / inputs ----
        cst = P.tile([128, 104], F32, tag="cst", name="cst")
        nc.sync.dma_start(out=cst[:], in_=consts[:])
        ident = P.tile([128, 128], F32, tag="ident", name="ident")
        make_identity(nc, ident[:])

        rgb = []
        for ch in range(3):
            t0 = P.tile([R0, W], F32, tag=f"rgb{ch}0", name=f"rgb{ch}0")
            t1 = P.tile([R1, W], F32, tag=f"rgb{ch}1", name=f"rgb{ch}1")
            nc.sync.dma_start(out=t0[:], in_=img[ch, 0:R0, :])
            nc.sync.dma_start(out=t1[:], in_=img[ch, R0:RL, :])
            rgb.append((t0, t1))
        poil_t0 = P.tile([R0, 1], F32, tag="poil0", name="poil0")
        poil_t1 = P.tile([R1, 1], F32, tag="poil1", name="poil1")
        nc.sync.dma_start(out=poil_t0[:], in_=poil[0:R0, :])
        nc.sync.dma_start(out=poil_t1[:], in_=poil[R0:RL, :])
        poic_t = P.tile([128, RC], F32, tag="poic", name="poic")
        nc.sync.dma_start(out=poic_t[:], in_=poic[:])

        # Toeplitz constants -> bf16 (DVE cast; DMA-produced f32 first)
        thl_f = [P.tile([R0, NPAD], F32, tag="thlf0", name="thlf0"),
                 P.tile([R1, NPAD], F32, tag="thlf1", name="thlf1")]
        nc.sync.dma_start(out=thl_f[0][:], in_=thl[0:R0, :])
        nc.sync.dma_start(out=thl_f[1][:], in_=thl[R0:RL, :])
        thl_r = [P.tile([R0, NPAD], BF16, tag="thlr0", name="thlr0"),
                 P.tile([R1, NPAD], BF16, tag="thlr1", name="thlr1")]
        nc.vector.tensor_copy(thl_r[0][:], thl_f[0][:])
        nc.vector.tensor_copy(thl_r[1][:], thl_f[1][:])

        twl_f = [P.tile([128, W], F32, tag=f"twlf{i}", name=f"twlf{i}") for i in range(4)]
        twl_r = [P.tile([128, W], BF16, tag=f"twlr{i}", name=f"twlr{i}") for i in range(4)]
        for i in range(4):
            nc.sync.dma_start(out=twl_f[i][:], in_=twl[128 * i:128 * (i + 1), :])
            nc.vector.tensor_copy(twl_r[i][:], twl_f[i][:])
        twc_f = [P.tile([128, W], F32, tag=f"twcf{i}", name=f"twcf{i}") for i in range(4)]
        twc_r = [P.tile([128, W], BF16, tag=f"twcr{i}", name=f"twcr{i}") for i in range(4)]
        for i in range(4):
            nc.sync.dma_start(out=twc_f[i][:], in_=twc[128 * i:128 * (i + 1), :])
            nc.vector.tensor_copy(twc_r[i][:], twc_f[i][:])
        thc_f = [P.tile([128, BAND], F32, tag="thcf0", name="thcf0"),
                 P.tile([RC - 128, BAND], F32, tag="thcf1", name="thcf1")]
        nc.sync.dma_start(out=thc_f[0][:], in_=thc[0:128, :])
        nc.sync.dma_start(out=thc_f[1][:], in_=thc[128:RC, :])
        thc_r = [P.tile([128, BAND], BF16, tag="thcr0", name="thcr0"),
                 P.tile([RC - 128, BAND], BF16, tag="thcr1", name="thcr1")]
        nc.vector.tensor_copy(thc_r[0][:], thc_f[0][:])
        nc.vector.tensor_copy(thc_r[1][:], thc_f[1][:])

        # ---- P1: YUV + z (y-part) ----
        lum = [P.tile([R0, W], F32, tag="lum0", name="lum0"), P.tile([R1, W], F32, tag="lum1", name="lum1")]
        uch = [P.tile([R0, W], F32, tag="u0", name="u0"), P.tile([R1, W], F32, tag="u1", name="u1")]
        vch = [P.tile([R0, W], F32, tag="v0", name="v0"), P.tile([R1, W], F32, tag="v1", name="v1")]
        for i in range(2):
            r_t, g_t, b_t = rgb[0][i], rgb[1][i], rgb[2][i]
            for dst, row in ((lum, 0), (uch, 1), (vch, 2)):
                c0, c1, c2 = (float(_RGB2YUV[row, 0]), float(_RGB2YUV[row, 1]),
                              float(_RGB2YUV[row, 2]))
                nc.vector.tensor_scalar_mul(dst[i][:], r_t[:], c0)
                nc.vector.scalar_tensor_tensor(dst[i][:], g_t[:], c1, dst[i][:],
                                               AL.mult, AL.add)
                nc.vector.scalar_tensor_tensor(dst[i][:], b_t[:], c2, dst[i][:],
                                               AL.mult, AL.add)

        z = [P.tile([R0, W], F32, tag="z0", name="z0"), P.tile([R1, W], F32, tag="z1", name="z1")]
        za = [P.tile([R0, W], F32, tag="za0", name="za0"), P.tile([R1, W], F32, tag="za1", name="za1")]
        za2 = [P.tile([R0, W], F32, tag="za20", name="za20"), P.tile([R1, W], F32, tag="za21", name="za21")]
        for i, poi_t in ((0, poil_t0), (1, poil_t1)):
            n = R0 if i == 0 else R1
            nc.vector.tensor_scalar(z[i][:], lum[i][:], 0.0, 1.0, AL.max, AL.min)
            nc.vector.tensor_scalar(z[i][:], z[i][:], float(D - 1), poi_t[:, 0:1],
                                    AL.mult, AL.add)
            # za = alpha_l * z ; za2 = alpha_l * z^2
            nc.vector.tensor_single_scalar(za[i][:], z[i][:], cst[:n, C_AL:C_AL + 1], AL.mult)
            nc.vector.tensor_mul(za2[i][:], za[i][:], z[i][:])

        lum_bf = [P.tile([R0, W], BF16, tag="lumbf0", name="lumbf0"),
                  P.tile([R1, W], BF16, tag="lumbf1", name="lumbf1")]
        nc.vector.tensor_copy(lum_bf[0][:], lum[0][:])
        nc.vector.tensor_copy(lum_bf[1][:], lum[1][:])

        # ---- transposes: zT, U_x, V_x (x-part, chrom rows) -> WIDE tiles ----
        zT = P.tile([128, 4 * RC], F32, tag="zT", name="zT")
        u_x = P.tile([128, 4 * RC], BF16, tag="ux", name="ux")
        v_x = P.tile([128, 4 * RC], BF16, tag="vx", name="vx")

        with ExitStack() as tctx:
            tp_ps = tctx.enter_context(tc.tile_pool(name="tp_ps", bufs=2, space="PSUM"))

            def transpose_chrom_rows(src_pair, dst_wide):
                for xc in range(4):
                    ps = tp_ps.tile([128, RL], F32, tag="tpps", name="tpps")
                    nc.tensor.transpose(ps[:, 0:R0],
                                        src_pair[0][:, 128 * xc:128 * (xc + 1)],
                                        ident[:])
                    nc.tensor.transpose(ps[:, R0:RL],
                                        src_pair[1][:, 128 * xc:128 * (xc + 1)],
                                        ident[0:R1, 0:R1])
                    dst = dst_wide[:, xc * RC:(xc + 1) * RC]
                    if xc % 2 == 0:
                        nc.vector.tensor_copy(dst, ps[:, CA0:CA0 + RC])
                    else:
                        nc.scalar.copy(dst, ps[:, CA0:CA0 + RC])

            transpose_chrom_rows(z, zT)
            transpose_chrom_rows(uch, u_x)
            transpose_chrom_rows(vch, v_x)

        # ---- P2: LUM bins ----
        # numden layout: [xc][ci=2][RC]  (ci=0 num, ci=1 den)
        numden_l = P.tile([128, 4 * 2 * RC], F32, tag="numden", name="numden")
        nc.vector.memset(numden_l[:], 0.0)

        with ExitStack() as lctx:
            LP = lctx.enter_context(tc.tile_pool(name="lum_loop", bufs=2))
            # conv1 psum: one [128, 1024] f32 (2 banks) per ci; 4 chunks at 256-slots
            ps1 = lctx.enter_context(tc.tile_pool(name="l_ps1", bufs=2, space="PSUM"))
            # conv2 psum: one [128, 1024] f32 (2 banks) per xo-pair; 2 per d
            ps2 = lctx.enter_context(tc.tile_pool(name="l_ps2", bufs=2, space="PSUM"))

            for d in range(D):
                # splat profile: exp(2d*za - za2 + expb_l[d])
                u2 = [LP.tile([R0, W], F32, tag="u2_0", name="u2_0"),
                      LP.tile([R1, W], F32, tag="u2_1", name="u2_1")]
                prof = [LP.tile([R0, W], BF16, tag="prof0", name="prof0"),
                        LP.tile([R1, W], BF16, tag="prof1", name="prof1")]
                gy = [LP.tile([R0, W], BF16, tag="gy0", name="gy0"),
                      LP.tile([R1, W], BF16, tag="gy1", name="gy1")]
                nc.vector.scalar_tensor_tensor(u2[0][:], za[0][:], float(2.0 * d),
                                               za2[0][:], AL.mult, AL.subtract)
                nc.vector.scalar_tensor_tensor(u2[1][:], za[1][:], float(2.0 * d),
                                               za2[1][:], AL.mult, AL.subtract)
                for i in range(2):
                    n = R0 if i == 0 else R1
                    nc.scalar.activation(prof[i][:], u2[i][:], AF.Exp,
                                         bias=cst[:n, C_EXPB_L + d:C_EXPB_L + d + 1],
                                         scale=1.0)
                    if i == 0:
                        nc.gpsimd.tensor_mul(gy[i][:], lum_bf[i][:], prof[i][:])
                    else:
                        nc.vector.tensor_mul(gy[i][:], lum_bf[i][:], prof[i][:])

                # conv1: H-conv fused transpose -> packed psum, one strided cast per ci
                # c1p layout: [xi][ci=2][RC] bf16
                c1p = LP.tile([128, 4 * 2 * RC], BF16, tag="l_c1p", name="l_c1p")
                for ci, ch in enumerate((gy, prof)):
                    pst = ps1.tile([128, 1024], F32, tag="l_c1ps", name="l_c1ps")
                    for xc in range(4):
                        dst = pst[:, xc * 256:xc * 256 + RC]
                        nc.tensor.matmul(dst, ch[0][:, 128 * xc:128 * (xc + 1)],
                                         thl_r[0][:, 0:RC], start=True, stop=False)
                        nc.tensor.matmul(dst, ch[1][:, 128 * xc:128 * (xc + 1)],
                                         thl_r[1][:, 0:RC], start=False, stop=True)
                    src = _ap(pst, 0, [[256, 4], [1, RC]])
                    dst = _ap(c1p, ci * RC, [[2 * RC, 4], [1, RC]])
                    if ci == 0:
                        nc.vector.tensor_copy(dst, src)
                    else:
                        nc.scalar.copy(dst, src)

                # conv2: W-conv (x-part); xo pairs into [128,1024] psum (2 banks)
                pst2 = []
                for pair in range(2):
                    pt = ps2.tile([128, 1024], F32, tag="l_c2ps", name="l_c2ps")
                    pst2.append(pt)
                    for sub in range(2):
                        xo = 2 * pair + sub
                        dst = pt[:, sub * 512:sub * 512 + 2 * RC]
                        first = True
                        xis = [x for x in (xo - 1, xo, xo + 1) if 0 <= x <= 3]
                        for xi in xis:
                            nc.tensor.matmul(
                                dst, twl_r[xi][:, 128 * xo:128 * (xo + 1)],
                                c1p[:, xi * 2 * RC:(xi + 1) * 2 * RC],
                                start=first, stop=(xi == xis[-1]))
                            first = False

                # slice: hat_d(zT) wide; mul reads PSUM directly; add on gpsimd
                hat = LP.tile([128, 4 * RC], F32, tag="l_hat", name="l_hat")
                hu = LP.tile([128, 4 * RC], F32, tag="l_hu", name="l_hu")
                nc.scalar.activation(hu[:], zT[:], AF.Abs,
                                     bias=cst[:, C_HATB + d:C_HATB + d + 1], scale=1.0)
                nc.scalar.activation(hat[:], hu[:], AF.Relu,
                                     bias=cst[:, C_ONE:C_ONE + 1], scale=-1.0)
                for pair in range(2):
                    pr = LP.tile([128, 2 * 2 * RC], F32, tag=f"l_pr{pair}", name=f"l_pr{pair}")
                    hat_b = _ap(hat, pair * 2 * RC, [[RC, 2], [0, 2], [1, RC]])
                    ps_r = _ap(pst2[pair], 0, [[512, 2], [RC, 2], [1, RC]])
                    nc.vector.tensor_mul(pr[:], hat_b, ps_r)
                    dst = numden_l[:, pair * 2 * 2 * RC:(pair + 1) * 2 * 2 * RC]
                    nc.gpsimd.tensor_add(dst, dst, pr[:])

        # ---- out_lum = num/max(den,eps); zc = clip(OL)*31 + poison ----
        # numden layout [xc][num RC | den RC]; build wide ol/zc [xc][RC]
        den_ap = _ap(numden_l, RC, [[2 * RC, 4], [1, RC]])
        nc.vector.tensor_single_scalar(den_ap, den_ap, cst[:, C_EPS:C_EPS + 1], AL.max)
        denr = P.tile([128, 4 * RC], F32, tag="denr", name="denr")
        nc.vector.reciprocal(denr[:], den_ap)
        ol = P.tile([128, 4 * RC], F32, tag="ol", name="ol")
        num_ap = _ap(numden_l, 0, [[2 * RC, 4], [1, RC]])
        nc.vector.tensor_mul(ol[:], num_ap, denr[:])
        zc = P.tile([128, 4 * RC], F32, tag="zc", name="zc")
        zca = P.tile([128, 4 * RC], F32, tag="zca", name="zca")
        zca2 = P.tile([128, 4 * RC], F32, tag="zca2", name="zca2")
        nc.vector.tensor_scalar(zc[:], ol[:], 0.0, 1.0, AL.max, AL.min)
        nc.vector.tensor_scalar_mul(zc[:], zc[:], float(D - 1))
        poic_b = _ap(poic_t, 0, [[0, 4], [1, RC]])
        nc.vector.tensor_add(zc[:], zc[:], poic_b)
        nc.vector.tensor_single_scalar(zca[:], zc[:], cst[:, C_AC:C_AC + 1], AL.mult)
        nc.vector.tensor_mul(zca2[:], zca[:], zc[:])

        # zcT: y-part zc at band rows (chrom window rows 30..158)
        zcT = P.tile([128, W], F32, tag="zcT", name="zcT")
        olb = P.tile([128, W], F32, tag="olb", name="olb")
        with ExitStack() as tctx2:
            tp2 = tctx2.enter_context(tc.tile_pool(name="tp_ps2", bufs=2, space="PSUM"))
            for xc in range(4):
                ps = tp2.tile([128, 128], F32, tag="tpps2", name="tpps2")
                nc.tensor.transpose(ps[:], zc[:, xc * RC + HALO_C:xc * RC + HALO_C + BAND], ident[:])
                nc.scalar.copy(zcT[:, 128 * xc:128 * (xc + 1)], ps[:])
                ps2b = tp2.tile([128, 128], F32, tag="tpps2", name="tpps2")
                nc.tensor.transpose(ps2b[:], ol[:, xc * RC + HALO_C:xc * RC + HALO_C + BAND], ident[:])
                nc.vector.tensor_copy(olb[:, 128 * xc:128 * (xc + 1)], ps2b[:])

        # ---- P4: CHROM bins ----
        acc_c = P.tile([128, 3 * W], F32, tag="accc", name="accc")
        nc.vector.memset(acc_c[:], 0.0)

        with ExitStack() as cctx:
            CP = cctx.enter_context(tc.tile_pool(name="ch_loop", bufs=2))
            cps1 = cctx.enter_context(tc.tile_pool(name="c_ps1", bufs=4, space="PSUM"))
            cps2 = cctx.enter_context(tc.tile_pool(name="c_ps2", bufs=1, space="PSUM"))

            YC0, YC1 = 128, RC - 128  # y chunks of the chrom window (128 + 60)
            for d in range(D):
                # wide splat: u2c/profc/gu/gv [128, 4*RC]
                u2c = CP.tile([128, 4 * RC], F32, tag="cu2", name="cu2")
                profc = CP.tile([128, 4 * RC], BF16, tag="cprof", name="cprof")
                gu = CP.tile([128, 4 * RC], BF16, tag="cgu", name="cgu")
                gv = CP.tile([128, 4 * RC], BF16, tag="cgv", name="cgv")
                nc.vector.scalar_tensor_tensor(u2c[:], zca[:], float(2.0 * d),
                                               zca2[:], AL.mult, AL.subtract)
                nc.scalar.activation(profc[:], u2c[:], AF.Exp,
                                     bias=cst[:, C_EXPB_C + d:C_EXPB_C + d + 1],
                                     scale=1.0)
                nc.vector.tensor_mul(gu[:], u_x[:], profc[:])
                nc.gpsimd.tensor_mul(gv[:], v_x[:], profc[:])

                # conv1: W-conv fused transpose -> [y, x_out 512]
                c1c = {}
                for ci, ch in enumerate((gu, gv, profc)):
                    for yc, (ys, nrows) in enumerate(((0, YC0), (YC0, YC1))):
                        ps = cps1.tile([128, W], F32, tag="c_c1ps", name="c_c1ps")
                        nc.tensor.matmul(ps[0:nrows, :], ch[:, ys:ys + nrows],
                                         twc_r[0][:], start=True, stop=False)
                        for xi in (1, 2, 3):
                            s = min(128 * xi - HALO_C, W - NPAD)
                            nc.tensor.matmul(
                                ps[0:nrows, s:s + NPAD],
                                ch[:, xi * RC + ys:xi * RC + ys + nrows],
                                twc_r[xi][:, s:s + NPAD],
                                start=False, stop=(xi == 3))
                        t = CP.tile([128, W], BF16, tag=f"c_c1_{ci}_{yc}", name=f"c_c1_{ci}_{yc}")
                        if (ci * 2 + yc) % 2 == 0:
                            nc.vector.tensor_copy(t[0:nrows, :], ps[0:nrows, :])
                        else:
                            nc.scalar.copy(t[0:nrows, :], ps[0:nrows, :])
                        c1c[(ci, yc)] = t

                # conv2: H-conv y-part -> [y_out 128, x 512] x3 into one psum tile
                pt = cps2.tile([128, 3 * W], F32, tag="c_c2ps", name="c_c2ps")
                for ci in range(3):
                    dst = pt[:, ci * W:(ci + 1) * W]
                    nc.tensor.matmul(dst, thc_r[0][:], c1c[(ci, 0)][0:YC0, :],
                                     start=True, stop=False)
                    nc.tensor.matmul(dst, thc_r[1][:], c1c[(ci, 1)][0:YC1, :],
                                     start=False, stop=True)

                # slice (y-part): hat wide; mul reads PSUM; add on gpsimd
                hat = CP.tile([128, W], F32, tag="c_hat", name="c_hat")
                hu = CP.tile([128, W], F32, tag="c_hu", name="c_hu")
                nc.scalar.activation(hu[:], zcT[:], AF.Abs,
                                     bias=cst[:, C_HATB + d:C_HATB + d + 1], scale=1.0)
                nc.scalar.activation(hat[:], hu[:], AF.Relu,
                                     bias=cst[:, C_ONE:C_ONE + 1], scale=-1.0)
                hat_b = _ap(hat, 0, [[0, 3], [1, W]])
                prc = CP.tile([128, 3 * W], F32, tag="c_prc", name="c_prc")
                nc.vector.tensor_mul(prc[:], hat_b, pt[:])
                nc.gpsimd.tensor_add(acc_c[:], acc_c[:], prc[:])

        # ---- P5: output ----
        den_v = acc_c[:, 2 * W:3 * W]
        nc.vector.tensor_single_scalar(den_v, den_v, cst[:, C_EPS:C_EPS + 1],
                                       AL.max)
        nc.vector.reciprocal(den_v, den_v)
        ocu = P.tile([128, W], F32, tag="ocu", name="ocu")
        ocv = P.tile([128, W], F32, tag="ocv", name="ocv")
        nc.vector.tensor_mul(ocu[:], acc_c[:, 0:W], den_v)
        nc.vector.tensor_mul(ocv[:], acc_c[:, W:2 * W], den_v)

        for ch in range(3):
            c0, c1_, c2 = (float(_YUV2RGB[ch, 0]), float(_YUV2RGB[ch, 1]),
                           float(_YUV2RGB[ch, 2]))
            o = P.tile([128, W], F32, tag=f"outc{ch}", name=f"outc{ch}")
            nc.vector.tensor_scalar_mul(o[:], olb[:], c0)
            nc.vector.scalar_tensor_tensor(o[:], ocu[:], c1_, o[:], AL.mult, AL.add)
            nc.vector.scalar_tensor_tensor(o[:], ocv[:], c2, o[:], AL.mult, AL.add)
            nc.sync.dma_start(out=out[ch, :, :], in_=o[:])

    print(f"build_nc: traced+scheduled in {_time.time()-_t0:.1f}s", flush=True)
    if waitsplit:
        _split_multi_waits(nc)
    return nc


def _split_multi_waits(nc, max_waits=1):
    counter = [0]
    for f in nc.m.functions:
        for blk in f.blocks:
            insts = blk.instructions
            snapshot = list(insts)
            offset = 0
            for idx, inst in enumerate(snapshot):
                si = inst.sync_info
                if si is None or not si.on_wait or len(si.on_wait) <= max_waits:
                    continue
                waits = list(si.on_wait)
                keep = waits[-max_waits:]
                extra = waits[:-max_waits]
                new_insts = []
                for wcond in extra:
                    ev = mybir.InstEventSemaphore(
                        name=f"I-wsplit-{counter[0]}", ins=[], outs=[])
                    counter[0] += 1
                    ev.engine = inst.engine
                    ev.sync_info = mybir.SyncInfo(on_wait=[wcond], on_update=[])
                    new_insts.append(ev)
                inst.sync_info = mybir.SyncInfo(on_wait=keep,
                                                on_update=list(si.on_update))
                for j, ev in enumerate(new_insts):
                    insts.insert(idx + offset + j, ev)
                offset += len(new_insts)


# ---------------------------------------------------------------------------
# host side
# ---------------------------------------------------------------------------

_NC_CACHE = None


def _get_nc():
    global _NC_CACHE
    if _NC_CACHE is None:
        _NC_CACHE = build_nc()
    return _NC_CACHE


def _make_in_maps(image, f_s, f_r, f_sc, f_rc):
    image = np.ascontiguousarray(np.asarray(image, np.float32))
    f_s = np.asarray(f_s, np.float32)
    f_sc = np.asarray(f_sc, np.float32)
    f_r = np.asarray(f_r, np.float64)
    f_rc = np.asarray(f_rc, np.float64)

    thl_m = _toeplitz(f_s, RL, RC, (len(f_s) - 1) // 2)
    thl_pad = np.zeros((RL, NPAD), np.float32)
    thl_pad[:, :RC] = thl_m
    twl_m = _toeplitz(f_s, W, W, 0)
    thc_m = _toeplitz(f_sc, RC, BAND, (len(f_sc) - 1) // 2)
    twc_m = _toeplitz(f_sc, W, W, 0)

    def alpha_lnc(k):
        c = (len(k) - 1) // 2
        alpha = float(np.log(k[c] / k[c + 1]))
        lnc = float(np.log(k[c]))
        return alpha, lnc

    al, lncl = alpha_lnc(f_r)
    ac, lncc = alpha_lnc(f_rc)

    consts = np.zeros((128, 104), np.float32)
    dd = np.arange(D, dtype=np.float64)
    consts[:, 0:32] = (lncl - al * dd * dd)[None, :]
    consts[:, 32:64] = (lncc - ac * dd * dd)[None, :]
    consts[:, 64:96] = (-dd)[None, :]
    consts[:, 96] = al
    consts[:, 97] = ac
    consts[:, 98] = 1.0
    consts[:, 99] = 1e-8

    in_maps = []
    for c in range(8):
        b, t = divmod(c, N_BANDS)
        s = t * BAND
        rows = np.arange(s - HALO_L, s + BAND + HALO_L)
        valid = (rows >= 0) & (rows < H)
        win = np.zeros((3, RL, W), np.float32)
        win[:, valid] = image[b][:, rows[valid]]
        poil_v = np.where(valid, 0.0, POISON).astype(np.float32)[:, None]
        validc = valid[HALO_L - HALO_C:HALO_L - HALO_C + RC]
        poic_v = np.broadcast_to(
            np.where(validc, 0.0, POISON).astype(np.float32)[None, :],
            (128, RC)).copy()
        in_maps.append({
            "img": win, "thl": thl_pad, "twl": twl_m, "thc": thc_m,
            "twc": twc_m, "consts": consts, "poil": poil_v, "poic": poic_v,
        })
    return in_maps


def kernel(image, filter_s, filter_r, filter_s_color, filter_r_color,
           _trace=False):
    nc = _get_nc()
    in_maps = _make_in_maps(image, filter_s, filter_r, filter_s_color,
                            filter_r_color)
    res = run_bass_kernel_spmd(nc, in_maps, list(range(8)), trace=_trace)
    out = np.zeros((BATCH, 3, H, W), np.float32)
    for c in range(8):
        b, t = divmod(c, N_BANDS)
        out[b, :, t * BAND:(t + 1) * BAND, :] = res.results[c]["out"]
    if _trace:
        return out, res
    return out
